# revision 79
# baseline (speedup 1.0000x reference)
"""Trainium2 Bass kernel for the BNN FASHION FC problem.

Network (per reference):
  h = x.reshape(B, 784)
  L1: h @ binarize(w1).T + b1 -> BN -> clip -> binarize     [B, 2048]
  L2: h @ binarize(w2).T + b2 -> BN -> clip -> binarize     [B, 2048]
  L3: (h @ binarize(w3).T + b3) * scale                     [B, 10]

Strategy (8 NeuronCores, data-parallel over batch, weights replicated):
  - Everything is computed with the hidden index on SBUF/PSUM partitions,
    so batchnorm affine + sign folds into one ScalarE activation (Sign
    with per-partition scale/bias) and layer N's output lands exactly in
    the [k=hidden, free=batch] layout layer N+1 needs.
  - The host-side sharding uploads x / w1 / w2 / w3 in contraction-major
    (transposed) layout, so layout prep on-device is DMA + Sign / hi-lo
    split only -- no PE transposes.  Binarization and the x split stay
    on device.
  - L1 splits x = hi + lo. hi runs in fp16 over 7 k-tiles, with the
    packed tail tile carrying hi rows 0:16 and an exact fp16 lo-tail in
    rows 16:32 (both against +-1 fp16 weights).  lo for the six full
    k-tiles runs in fp8e4m3 scaled by 2^14 against +-2^-14 fp8e5m2
    weights in DoubleRow mode (3 matmuls).  10 matmuls/tile total vs 13
    for the exact fp16 hi/lo scheme; final rel err ~1.5e-2 (gate 2e-2).
  - L2/L3 operands are all +-1: exact in fp8e4, run with DoubleRow.
  - binarize(clip(y)) == binarize(y), so clip is dropped.
  - BN folding: y = mm*inv + c with inv = g/sqrt(v+eps), c = (b-m)*inv + be.
  - L3 DoubleRow matmuls are interleaved into the L2 o-loop (lagged so
    the Sign drains stay off the PE critical path), shortening the tail.

Output per core is [10, 2048] (hidden-major); the host transposes and
concatenates to the full [16384, 10].
"""

import numpy as np
from contextlib import ExitStack

try:
    import concourse.bass as bass
except ImportError:  # staged repo location
    import sys

    sys.path.insert(0, "/opt/trn_rl_repo")
    import concourse.bass as bass

import concourse.mybir as mybir
import concourse.tile as tile
from concourse import bacc
from concourse.bass_utils import run_bass_kernel_spmd


P = 128
N_CORES = 8
B = 16384
B_LOC = B // N_CORES  # 2048 batch rows per core
D_IN = 784
KT = 7  # k tiles for layer-1 hi pass (784 -> 896 with packed lo-tail)
KLO = 6  # k tiles for layer-1 fp8 lo pass (the 768 full columns)
DP = KT * P  # 896
H = 2048
HS = H // P  # 16 hidden subtiles
NF = 512  # matmul free dim (one PSUM bank)
NB = B_LOC // NF  # 4 batch chunks
O = 10
KTAIL = D_IN - KLO * P  # 16-row tail (784 = 6*128 + 16)
LO_SCALE = float(2.0**14)  # lo pass: data scaled up, weights scaled down
W_LO = float(2.0**-14)  # exactly representable (normal) in fp8e5m2

F32 = mybir.dt.float32
F16 = mybir.dt.float16
F8 = mybir.dt.float8e4
F8E5 = mybir.dt.float8e5
AF = mybir.ActivationFunctionType
ALU = mybir.AluOpType
DR = mybir.MatmulPerfMode.DoubleRow


def _build():
    nc = bacc.Bacc(trn_type="TRN2")

    def din(name, shape):
        return nc.dram_tensor(name, shape, F32, kind="ExternalInput")

    # x/w1/w2/w3 arrive contraction-major (pre-transposed by the host shard)
    x = din("x", [D_IN, B_LOC])
    w1 = din("w1", [D_IN, H])
    b1 = din("b1", [H])
    g1 = din("g1", [H])
    be1 = din("be1", [H])
    m1 = din("m1", [H])
    v1 = din("v1", [H])
    w2 = din("w2", [H, H])
    b2 = din("b2", [H])
    g2 = din("g2", [H])
    be2 = din("be2", [H])
    m2 = din("m2", [H])
    v2 = din("v2", [H])
    w3 = din("w3", [H, O])
    b3 = din("b3", [O])
    scale = din("scale", [1])
    out = nc.dram_tensor("out", [O, B_LOC], F32, kind="ExternalOutput")

    with ExitStack() as ctx:
        tc = ctx.enter_context(tile.TileContext(nc))
        consts = ctx.enter_context(tc.tile_pool(name="consts", bufs=1))
        big = ctx.enter_context(tc.tile_pool(name="big", bufs=1))
        stage = ctx.enter_context(tc.tile_pool(name="stage", bufs=2))
        wstage = ctx.enter_context(tc.tile_pool(name="wstage", bufs=2))
        psum = ctx.enter_context(tc.tile_pool(name="psum", bufs=1, space="PSUM"))

        # PE warm-up: dependency-free dummy matmuls (outputs never read).
        # They run right after the engine barrier while the first x/w1 tiles
        # are still loading, so the HAM clock gate is already at 8/8 when
        # the real matmuls arrive.
        warm_in = consts.tile([P, NF], F16, name="warm_in")
        nc.vector.memset(warm_in[:], 1.0)
        warm_ps = psum.tile([P, NF], F32, tag="l3", bufs=1, name="warm_ps")

        def warm(k, w=NF):
            for _ in range(k):
                nc.tensor.matmul(
                    warm_ps[:, :w], warm_in[:, :P], warm_in[:, :w], start=True, stop=True
                )

        warm(14)

        # ---- persistent big tensors ----
        # h2b shares the xThi slot (xThi dead once L1 is done); xThi padded
        # to 8 k-tiles so the slot is h2b-sized.
        xThi = big.tile([P, 8, B_LOC], F16, tag="bigA", name="xThi")
        xTlo = big.tile([P, KLO, B_LOC], F8, tag="bigB", name="xTlo")
        w1bT = big.tile([P, KT, H], F16, tag="bigC", name="w1bT")
        w1lT = big.tile([P, KLO, H], F8E5, tag="bigD", name="w1lT")
        w2bT = big.tile([P, HS, H], F8, tag="bigE", name="w2bT")
        h1b = big.tile([P, HS, B_LOC], F8, tag="bigF", name="h1b")

        # the packed-tail k-tile: rows 0:16 hi, 16:32 lo16, rest must be 0
        nc.vector.memset(xThi[:, KLO, :], 0.0)
        nc.vector.memset(w1bT[:, KLO, :], 0.0)

        def w1_prep(ht):
            """Load the [784, 128] w1 h-slice (contraction-major), binarize
            into w1bT (with the fp16 lo-tail weights duplicated in rows
            16:32 of k-tile 6) and derive the +-2^-14 fp8e5 lo weights."""
            hsl = slice(ht * P, (ht + 1) * P)
            w1sb = stage.tile([P, KLO, P], F32, tag="w1f32", bufs=2, name="w1sb")
            nc.sync.dma_start(
                w1sb[:], w1[: KLO * P, hsl].rearrange("(k p) h -> p k h", p=P)
            )
            nc.scalar.activation(w1bT[:, :KLO, hsl], w1sb[:], AF.Sign)
            nc.vector.tensor_scalar(
                w1lT[:, :, hsl], w1bT[:, :KLO, hsl], W_LO, None, ALU.mult
            )
            wts = stage.tile([KTAIL, P], F32, tag="w1tl", bufs=2, name="wts")
            nc.gpsimd.dma_start(wts[:], w1[KLO * P : D_IN, hsl])
            nc.scalar.activation(w1bT[:KTAIL, KLO, hsl], wts[:], AF.Sign)
            nc.scalar.activation(w1bT[32 : 32 + KTAIL, KLO, hsl], wts[:], AF.Sign)

        def x_prep(n, g):
            """Load one 3-k-tile group of a batch chunk of the contraction-
            major x, split hi (fp16) / lo (fp8e4 scaled 2^14) on DVE; group 1
            also handles the 16-row tail. No PE work at all."""
            nsl = slice(n * NF, (n + 1) * NF)
            ksl = slice(3 * g, 3 * g + 3)
            xsb = stage.tile([P, 3, NF], F32, tag="xf32", bufs=2, name="xsb")
            nc.sync.dma_start(
                xsb[:],
                x[3 * g * P : (3 * g + 3) * P, nsl].rearrange(
                    "(k p) b -> p k b", p=P
                ),
            )
            hi = xThi[:, ksl, nsl]
            nc.vector.tensor_copy(hi, xsb[:])
            t16 = stage.tile([P, 3, NF], F16, tag="xlo16", bufs=1, name="t16")
            nc.vector.tensor_tensor(t16[:], xsb[:], hi, ALU.subtract)
            nc.vector.tensor_scalar(
                xTlo[:, ksl, nsl], t16[:], LO_SCALE, None, ALU.mult
            )
            if g == 1:
                xts = stage.tile([KTAIL, NF], F32, tag="xtl", bufs=2, name="xts")
                nc.scalar.dma_start(xts[:], x[KLO * P : D_IN, nsl])
                hit = xThi[:KTAIL, KLO, nsl]
                nc.scalar.copy(hit, xts[:])
                nc.vector.tensor_tensor(
                    xThi[32 : 32 + KTAIL, KLO, nsl], xts[:], hit, ALU.subtract
                )

        w2_tiles = {}

        def w2_dma(kt):
            w2sb = wstage.tile([P, H], F32, tag="w2f32", bufs=3, name="w2sb")
            nc.sync.dma_start(w2sb[:], w2[kt * P : (kt + 1) * P, :])
            w2_tiles[kt] = w2sb

        def w2_sign(kt):
            # binarize on the (idle) DVE to keep ScalarE free for L1 drains:
            # (w >= 0)*2 in fp8, then -1 in place -> +-1
            t = w2_tiles.pop(kt)
            nc.vector.tensor_scalar(
                w2bT[:, kt, :], t[:], 0.0, 2.0, ALU.is_ge, ALU.mult
            )
            nc.vector.tensor_scalar_add(w2bT[:, kt, :], w2bT[:, kt, :], -1.0)

        def l1_mm(n, h):
            nsl = slice(n * NF, (n + 1) * NF)
            pmm = psum.tile([P, NF], F32, tag="mm", bufs=5, name="pmm")
            hsl = slice(h * P, (h + 1) * P)
            for k in range(KT):
                nc.tensor.matmul(
                    pmm[:], w1bT[:, k, hsl], xThi[:, k, nsl], start=(k == 0), stop=False
                )
            for kk in range(KLO // 2):
                ksl = slice(2 * kk, 2 * kk + 2)
                nc.tensor.matmul(
                    pmm[:],
                    w1lT[:, ksl, hsl],
                    xTlo[:, ksl, nsl],
                    start=False,
                    stop=(kk == KLO // 2 - 1),
                    perf_mode=DR,
                )
            nc.scalar.activation(
                h1b[:, h, nsl],
                pmm[:],
                AF.Sign,
                bias=c1[:, h : h + 1],
                scale=inv1[:, h : h + 1],
            )

        # BN-fold constant tiles are pre-declared so l1_mm drains can
        # reference them; layer-2 constants fill later in the pipeline.
        inv1 = consts.tile([P, HS], F32, name="inv1")
        c1 = consts.tile([P, HS], F32, name="c1")
        inv2 = consts.tile([P, HS], F32, name="inv2")
        c2 = consts.tile([P, HS], F32, name="c2")
        b3sb = consts.tile([O, 1], F32, name="b3sb")
        s10 = consts.tile([O, 1], F32, name="s10")
        w3bT = consts.tile([P, HS, 16], F8, name="w3bT")
        w3sb = consts.tile([P, HS, O], F32, name="w3sb")

        # ---- per-hidden-unit BN constants, laid out [p, subtile] ----
        def vec_sb(handle, name):
            tmp = stage.tile([HS, P], F32, tag="vtmp", bufs=2, name="vtmp")
            nc.gpsimd.dma_start(tmp[:], handle[:].rearrange("(s p) -> s p", p=P))
            ps = psum.tile([P, HS], F32, tag="tr", bufs=2, name="vtps")
            nc.tensor.transpose(ps[:], tmp[:], id32[:])
            t = consts.tile([P, HS], F32, name=name)
            nc.vector.tensor_copy(t[:], ps[:])
            return t

        def bn_fold(inv, c, gs, bes, ms, bs, vs):
            nc.vector.tensor_scalar_add(inv, vs, 1e-5)
            nc.scalar.activation(inv, inv, AF.Sqrt)
            nc.vector.reciprocal(inv, inv)
            nc.vector.tensor_mul(inv, gs, inv)
            nc.vector.tensor_sub(c, bs, ms)
            nc.vector.tensor_mul(c, c, inv)
            nc.vector.tensor_add(c, c, bes)

        def const_fill1():
            b1s = vec_sb(b1, "b1s")
            g1s = vec_sb(g1, "g1s")
            be1s = vec_sb(be1, "be1s")
            m1s = vec_sb(m1, "m1s")
            v1s = vec_sb(v1, "v1s")
            bn_fold(inv1, c1, g1s, be1s, m1s, b1s, v1s)

        def const_fill2():
            b2s = vec_sb(b2, "b2s")
            g2s = vec_sb(g2, "g2s")
            be2s = vec_sb(be2, "be2s")
            m2s = vec_sb(m2, "m2s")
            v2s = vec_sb(v2, "v2s")
            bn_fold(inv2, c2, g2s, be2s, m2s, b2s, v2s)
            nc.gpsimd.dma_start(b3sb[:], b3[:].rearrange("(o u) -> o u", u=1))
            for i in range(O):
                nc.gpsimd.dma_start(
                    s10[i : i + 1, :], scale[:].rearrange("(s u) -> s u", u=1)
                )

        # identity for the small BN-vector transposes
        from concourse.masks import make_identity

        id32 = consts.tile([HS, HS], F32, name="id32")
        make_identity(nc, id32)

        # ---- main pipeline over batch chunks, software-pipelined prep ----
        w1_prep(0)
        x_prep(0, 0)
        warm(4, w=P)
        x_prep(0, 1)
        warm(4, w=P)
        for t in range(1, 4):
            w1_prep(t)
            warm(3, w=P)
        const_fill1()
        for n in range(NB):
            for h in range(HS):
                if n == 0 and h < 12:
                    w1_prep(4 + h)
                if h in (2, 4) and n + 1 < NB:
                    x_prep(n + 1, h // 2 - 1)
                if n == 0 and h in (13, 15):
                    w2_dma((h - 13) // 2)
                if n >= 1 and h % 2 == 1:
                    kt = 8 * (n - 1) + h // 2
                    if kt < HS:
                        w2_sign(kt)
                    if kt + 2 < HS:
                        w2_dma(kt + 2)
                l1_mm(n, h)
                if n == 0 and h == 2:
                    const_fill2()
                if n == 2 and h == 8:
                    # w3 contraction-major load (80KB, strided; hidden in
                    # L1 slack on the idle gpsimd queue)
                    nc.gpsimd.dma_start(
                        w3sb[:], w3[:].rearrange("(k p) o -> p k o", p=P)
                    )

        # ---- w3 binarize (load already done) ----
        nc.scalar.activation(w3bT[:, :, :O], w3sb[:], AF.Sign)

        # ---- layer 2 with layer-3 DoubleRow matmuls interleaved (lagged
        # two o-tiles so the Sign drains stay off the PE critical path) ----
        h2b = big.tile([P, HS, B_LOC], F8, tag="bigA", name="h2b")
        for n in range(NB):
            nsl = slice(n * NF, (n + 1) * NF)
            p3 = psum.tile([P, NF], F32, tag="l3", bufs=1, name="p3")

            def l3_pair(kk, n=n, nsl=nsl, p3=p3):
                ksl = slice(2 * kk, 2 * kk + 2)
                nc.tensor.matmul(
                    p3[:O, :],
                    w3bT[:, ksl, :O],
                    h2b[:, ksl, nsl],
                    start=(kk == 0),
                    stop=(kk == HS // 2 - 1),
                    perf_mode=DR,
                )

            for o in range(HS):
                if o >= 3 and o % 2 == 1:
                    l3_pair((o - 3) // 2)
                osl = slice(o * P, (o + 1) * P)
                pmm = psum.tile([P, NF], F32, tag="mm", bufs=5, name="pmm")
                for kk in range(HS // 2):
                    ksl = slice(2 * kk, 2 * kk + 2)
                    nc.tensor.matmul(
                        pmm[:],
                        w2bT[:, ksl, osl],
                        h1b[:, ksl, nsl],
                        start=(kk == 0),
                        stop=(kk == HS // 2 - 1),
                        perf_mode=DR,
                    )
                nc.scalar.activation(
                    h2b[:, o, nsl],
                    pmm[:],
                    AF.Sign,
                    bias=c2[:, o : o + 1],
                    scale=inv2[:, o : o + 1],
                )
            l3_pair(7)
            outsb = stage.tile([O, NF], F32, tag="outsb", name="outsb")
            nc.vector.tensor_scalar(
                outsb[:], p3[:O, :], b3sb[:], s10[:], ALU.add, ALU.mult
            )
            nc.sync.dma_start(out[:, nsl], outsb[:])

    nc.finalize()
    return nc


_CACHE = {}


def _get_nc():
    if "nc" not in _CACHE:
        _CACHE["nc"] = _build()
    return _CACHE["nc"]


def _in_maps(x, w1, b1, g1, be1, m1, v1, w2, b2, g2, be2, m2, v2, w3, b3, scale):
    f = lambda a: np.ascontiguousarray(np.asarray(a, dtype=np.float32))
    x2 = np.asarray(x, dtype=np.float32).reshape(B, D_IN)
    base = {
        "w1": f(np.asarray(w1, dtype=np.float32).T),
        "b1": f(b1),
        "g1": f(g1),
        "be1": f(be1),
        "m1": f(m1),
        "v1": f(v1),
        "w2": f(np.asarray(w2, dtype=np.float32).T),
        "b2": f(b2),
        "g2": f(g2),
        "be2": f(be2),
        "m2": f(m2),
        "v2": f(v2),
        "w3": f(np.asarray(w3, dtype=np.float32).T),
        "b3": f(b3),
        "scale": f(scale).reshape(1),
    }
    maps = []
    for c in range(N_CORES):
        m = dict(base)
        m["x"] = np.ascontiguousarray(x2[c * B_LOC : (c + 1) * B_LOC].T)
        maps.append(m)
    return maps


def _ensure_ntff_hook():
    """The agent image's antenv package lacks axon_hooks; synthesize it so
    run_bass_kernel_spmd's trace path can reach the axon NTFF profiler."""
    import sys
    import types

    if "antenv.axon_hooks" in sys.modules:
        return
    mod = types.ModuleType("antenv.axon_hooks")
    mod._hook = None

    def set_axon_ntff_profile_hook(h):
        mod._hook = h

    def get_axon_ntff_profile_hook():
        return mod._hook

    mod.set_axon_ntff_profile_hook = set_axon_ntff_profile_hook
    mod.get_axon_ntff_profile_hook = get_axon_ntff_profile_hook
    sys.modules["antenv.axon_hooks"] = mod
    import antenv

    antenv.axon_hooks = mod
    try:
        from trn_agent_boot.trn_boot import _ntff_profile_via_ctypes

        mod._hook = _ntff_profile_via_ctypes("/opt/axon/libaxon_pjrt.so")
    except Exception as e:
        print(f"ntff hook unavailable: {e}")


def run(trace=False, **inputs):
    if trace:
        _ensure_ntff_hook()
    nc = _get_nc()
    res = run_bass_kernel_spmd(
        nc, _in_maps(**inputs), core_ids=list(range(N_CORES)), trace=trace
    )
    outs = [r["out"] for r in res.results]
    full = np.concatenate([o.T for o in outs], axis=0).astype(np.float32)
    return full, res


def kernel(**inputs):
    return run(trace=False, **inputs)[0]


# revision 80
# speedup vs baseline: 1.0088x; 1.0088x over previous
"""Trainium2 Bass kernel for the BNN FASHION FC problem.

Network (per reference):
  h = x.reshape(B, 784)
  L1: h @ binarize(w1).T + b1 -> BN -> clip -> binarize     [B, 2048]
  L2: h @ binarize(w2).T + b2 -> BN -> clip -> binarize     [B, 2048]
  L3: (h @ binarize(w3).T + b3) * scale                     [B, 10]

Strategy (8 NeuronCores, data-parallel over batch, weights replicated):
  - Everything is computed with the hidden index on SBUF/PSUM partitions,
    so batchnorm affine + sign folds into one ScalarE activation (Sign
    with per-partition scale/bias) and layer N's output lands exactly in
    the [k=hidden, free=batch] layout layer N+1 needs.
  - The host-side sharding uploads x / w1 / w2 / w3 in contraction-major
    (transposed) layout, so layout prep on-device is DMA + Sign / hi-lo
    split only -- no PE transposes.  Binarization and the x split stay
    on device.
  - L1 splits x = hi + lo. hi runs in fp16 over 7 k-tiles, with the
    packed tail tile carrying hi rows 0:16 and an exact fp16 lo-tail in
    rows 16:32 (both against +-1 fp16 weights).  lo for the six full
    k-tiles runs in fp8e4m3 scaled by 2^14 against +-2^-14 fp8e5m2
    weights in DoubleRow mode (3 matmuls).  10 matmuls/tile total vs 13
    for the exact fp16 hi/lo scheme; final rel err ~1.5e-2 (gate 2e-2).
  - L2/L3 operands are all +-1: exact in fp8e4, run with DoubleRow.
  - binarize(clip(y)) == binarize(y), so clip is dropped.
  - BN folding: y = mm*inv + c with inv = g/sqrt(v+eps), c = (b-m)*inv + be.
  - L3 DoubleRow matmuls are interleaved into the L2 o-loop (lagged so
    the Sign drains stay off the PE critical path), shortening the tail.

Output per core is [10, 2048] (hidden-major); the host transposes and
concatenates to the full [16384, 10].
"""

import numpy as np
from contextlib import ExitStack

try:
    import concourse.bass as bass
except ImportError:  # staged repo location
    import sys

    sys.path.insert(0, "/opt/trn_rl_repo")
    import concourse.bass as bass

import concourse.mybir as mybir
import concourse.tile as tile
from concourse import bacc
from concourse.bass_utils import run_bass_kernel_spmd


P = 128
N_CORES = 8
B = 16384
B_LOC = B // N_CORES  # 2048 batch rows per core
D_IN = 784
KT = 7  # k tiles for layer-1 hi pass (784 -> 896 with packed lo-tail)
KLO = 6  # k tiles for layer-1 fp8 lo pass (the 768 full columns)
DP = KT * P  # 896
H = 2048
HS = H // P  # 16 hidden subtiles
NF = 512  # matmul free dim (one PSUM bank)
NB = B_LOC // NF  # 4 batch chunks
O = 10
KTAIL = D_IN - KLO * P  # 16-row tail (784 = 6*128 + 16)
LO_SCALE = float(2.0**14)  # lo pass: data scaled up, weights scaled down
W_LO = float(2.0**-14)  # exactly representable (normal) in fp8e5m2

F32 = mybir.dt.float32
F16 = mybir.dt.float16
F8 = mybir.dt.float8e4
F8E5 = mybir.dt.float8e5
AF = mybir.ActivationFunctionType
ALU = mybir.AluOpType
DR = mybir.MatmulPerfMode.DoubleRow


def _build():
    nc = bacc.Bacc(trn_type="TRN2")

    def din(name, shape):
        return nc.dram_tensor(name, shape, F32, kind="ExternalInput")

    # x/w1/w2/w3 arrive contraction-major (pre-transposed by the host shard)
    x = din("x", [D_IN, B_LOC])
    w1 = din("w1", [D_IN, H])
    b1 = din("b1", [H])
    g1 = din("g1", [H])
    be1 = din("be1", [H])
    m1 = din("m1", [H])
    v1 = din("v1", [H])
    w2 = din("w2", [H, H])
    b2 = din("b2", [H])
    g2 = din("g2", [H])
    be2 = din("be2", [H])
    m2 = din("m2", [H])
    v2 = din("v2", [H])
    w3 = din("w3", [H, O])
    b3 = din("b3", [O])
    scale = din("scale", [1])
    out = nc.dram_tensor("out", [O, B_LOC], F32, kind="ExternalOutput")

    with ExitStack() as ctx:
        tc = ctx.enter_context(tile.TileContext(nc))
        consts = ctx.enter_context(tc.tile_pool(name="consts", bufs=1))
        big = ctx.enter_context(tc.tile_pool(name="big", bufs=1))
        stage = ctx.enter_context(tc.tile_pool(name="stage", bufs=2))
        wstage = ctx.enter_context(tc.tile_pool(name="wstage", bufs=2))
        psum = ctx.enter_context(tc.tile_pool(name="psum", bufs=1, space="PSUM"))

        # PE warm-up: dependency-free dummy matmuls (outputs never read).
        # They run right after the engine barrier while the first x/w1 tiles
        # are still loading, so the HAM clock gate is already at 8/8 when
        # the real matmuls arrive.
        warm_in = consts.tile([P, NF], F16, name="warm_in")
        nc.vector.memset(warm_in[:], 1.0)
        warm_ps = psum.tile([P, NF], F32, tag="mm", bufs=5, name="warm_ps")

        def warm(k, w=NF):
            for _ in range(k):
                nc.tensor.matmul(
                    warm_ps[:, :w], warm_in[:, :P], warm_in[:, :w], start=True, stop=True
                )

        warm(14)

        # ---- persistent big tensors ----
        # h2b shares the xThi slot (xThi dead once L1 is done); xThi padded
        # to 8 k-tiles so the slot is h2b-sized.
        xThi = big.tile([P, 8, B_LOC], F16, tag="bigA", name="xThi")
        xTlo = big.tile([P, KLO, B_LOC], F8, tag="bigB", name="xTlo")
        w1bT = big.tile([P, KT, H], F16, tag="bigC", name="w1bT")
        w1lT = big.tile([P, KLO, H], F8E5, tag="bigD", name="w1lT")
        w2bT = big.tile([P, HS, H], F8, tag="bigE", name="w2bT")
        h1b = big.tile([P, HS, B_LOC], F8, tag="bigF", name="h1b")

        # the packed-tail k-tile: rows 0:16 hi, 16:32 lo16, rest must be 0
        nc.vector.memset(xThi[:, KLO, :], 0.0)
        nc.vector.memset(w1bT[:, KLO, :], 0.0)

        def w1_prep(ht):
            """Load the [784, 128] w1 h-slice (contraction-major), binarize
            into w1bT (with the fp16 lo-tail weights duplicated in rows
            16:32 of k-tile 6) and derive the +-2^-14 fp8e5 lo weights."""
            hsl = slice(ht * P, (ht + 1) * P)
            w1sb = stage.tile([P, KLO, P], F32, tag="w1f32", bufs=2, name="w1sb")
            nc.sync.dma_start(
                w1sb[:], w1[: KLO * P, hsl].rearrange("(k p) h -> p k h", p=P)
            )
            nc.scalar.activation(w1bT[:, :KLO, hsl], w1sb[:], AF.Sign)
            nc.vector.tensor_scalar(
                w1lT[:, :, hsl], w1bT[:, :KLO, hsl], W_LO, None, ALU.mult
            )
            wts = stage.tile([KTAIL, P], F32, tag="w1tl", bufs=2, name="wts")
            nc.gpsimd.dma_start(wts[:], w1[KLO * P : D_IN, hsl])
            nc.scalar.activation(w1bT[:KTAIL, KLO, hsl], wts[:], AF.Sign)
            nc.scalar.activation(w1bT[32 : 32 + KTAIL, KLO, hsl], wts[:], AF.Sign)

        def x_prep(n, g):
            """Load one 3-k-tile group of a batch chunk of the contraction-
            major x, split hi (fp16) / lo (fp8e4 scaled 2^14) on DVE; group 1
            also handles the 16-row tail. No PE work at all."""
            nsl = slice(n * NF, (n + 1) * NF)
            ksl = slice(3 * g, 3 * g + 3)
            xsb = stage.tile([P, 3, NF], F32, tag="xf32", bufs=2, name="xsb")
            nc.sync.dma_start(
                xsb[:],
                x[3 * g * P : (3 * g + 3) * P, nsl].rearrange(
                    "(k p) b -> p k b", p=P
                ),
            )
            hi = xThi[:, ksl, nsl]
            nc.vector.tensor_copy(hi, xsb[:])
            t16 = stage.tile([P, 3, NF], F16, tag="xlo16", bufs=1, name="t16")
            nc.vector.tensor_tensor(t16[:], xsb[:], hi, ALU.subtract)
            nc.vector.tensor_scalar(
                xTlo[:, ksl, nsl], t16[:], LO_SCALE, None, ALU.mult
            )
            if g == 1:
                xts = stage.tile([KTAIL, NF], F32, tag="xtl", bufs=2, name="xts")
                nc.scalar.dma_start(xts[:], x[KLO * P : D_IN, nsl])
                hit = xThi[:KTAIL, KLO, nsl]
                nc.scalar.copy(hit, xts[:])
                nc.vector.tensor_tensor(
                    xThi[32 : 32 + KTAIL, KLO, nsl], xts[:], hit, ALU.subtract
                )

        w2_tiles = {}

        def w2_dma(kt):
            w2sb = wstage.tile([P, H], F32, tag="w2f32", bufs=3, name="w2sb")
            nc.sync.dma_start(w2sb[:], w2[kt * P : (kt + 1) * P, :])
            w2_tiles[kt] = w2sb

        def w2_sign(kt):
            # binarize on the (idle) DVE to keep ScalarE free for L1 drains:
            # (w >= 0)*2 in fp8, then -1 in place -> +-1
            t = w2_tiles.pop(kt)
            nc.vector.tensor_scalar(
                w2bT[:, kt, :], t[:], 0.0, 2.0, ALU.is_ge, ALU.mult
            )
            nc.vector.tensor_scalar_add(w2bT[:, kt, :], w2bT[:, kt, :], -1.0)

        def l1_mm(n, h):
            nsl = slice(n * NF, (n + 1) * NF)
            pmm = psum.tile([P, NF], F32, tag="mm", bufs=5, name="pmm")
            hsl = slice(h * P, (h + 1) * P)
            for k in range(KT):
                nc.tensor.matmul(
                    pmm[:], w1bT[:, k, hsl], xThi[:, k, nsl], start=(k == 0), stop=False
                )
            for kk in range(KLO // 2):
                ksl = slice(2 * kk, 2 * kk + 2)
                nc.tensor.matmul(
                    pmm[:],
                    w1lT[:, ksl, hsl],
                    xTlo[:, ksl, nsl],
                    start=False,
                    stop=(kk == KLO // 2 - 1),
                    perf_mode=DR,
                )
            nc.scalar.activation(
                h1b[:, h, nsl],
                pmm[:],
                AF.Sign,
                bias=c1[:, h : h + 1],
                scale=inv1[:, h : h + 1],
            )

        # BN-fold constant tiles are pre-declared so l1_mm drains can
        # reference them; layer-2 constants fill later in the pipeline.
        inv1 = consts.tile([P, HS], F32, name="inv1")
        c1 = consts.tile([P, HS], F32, name="c1")
        inv2 = consts.tile([P, HS], F32, name="inv2")
        c2 = consts.tile([P, HS], F32, name="c2")
        b3sb = consts.tile([O, 1], F32, name="b3sb")
        s10 = consts.tile([O, 1], F32, name="s10")
        w3bT = consts.tile([P, HS, 16], F8, name="w3bT")
        w3sb = consts.tile([P, HS, O], F32, name="w3sb")

        # ---- per-hidden-unit BN constants, laid out [p, subtile] ----
        def vec_sb(handle, name):
            tmp = stage.tile([HS, P], F32, tag="vtmp", bufs=2, name="vtmp")
            nc.gpsimd.dma_start(tmp[:], handle[:].rearrange("(s p) -> s p", p=P))
            ps = psum.tile([P, HS], F32, tag="tr", bufs=2, name="vtps")
            nc.tensor.transpose(ps[:], tmp[:], id32[:])
            t = consts.tile([P, HS], F32, name=name)
            nc.vector.tensor_copy(t[:], ps[:])
            return t

        def bn_fold(inv, c, gs, bes, ms, bs, vs):
            nc.vector.tensor_scalar_add(inv, vs, 1e-5)
            nc.scalar.activation(inv, inv, AF.Sqrt)
            nc.vector.reciprocal(inv, inv)
            nc.vector.tensor_mul(inv, gs, inv)
            nc.vector.tensor_sub(c, bs, ms)
            nc.vector.tensor_mul(c, c, inv)
            nc.vector.tensor_add(c, c, bes)

        def const_fill1():
            b1s = vec_sb(b1, "b1s")
            g1s = vec_sb(g1, "g1s")
            be1s = vec_sb(be1, "be1s")
            m1s = vec_sb(m1, "m1s")
            v1s = vec_sb(v1, "v1s")
            bn_fold(inv1, c1, g1s, be1s, m1s, b1s, v1s)

        def const_fill2():
            b2s = vec_sb(b2, "b2s")
            g2s = vec_sb(g2, "g2s")
            be2s = vec_sb(be2, "be2s")
            m2s = vec_sb(m2, "m2s")
            v2s = vec_sb(v2, "v2s")
            bn_fold(inv2, c2, g2s, be2s, m2s, b2s, v2s)
            nc.gpsimd.dma_start(b3sb[:], b3[:].rearrange("(o u) -> o u", u=1))
            for i in range(O):
                nc.gpsimd.dma_start(
                    s10[i : i + 1, :], scale[:].rearrange("(s u) -> s u", u=1)
                )

        # identity for the small BN-vector transposes
        from concourse.masks import make_identity

        id32 = consts.tile([HS, HS], F32, name="id32")
        make_identity(nc, id32)

        # ---- main pipeline over batch chunks, software-pipelined prep ----
        w1_prep(0)
        x_prep(0, 0)
        warm(4, w=P)
        x_prep(0, 1)
        warm(4, w=P)
        for t in range(1, 4):
            w1_prep(t)
            warm(3, w=P)
        const_fill1()
        for n in range(NB):
            for h in range(HS):
                if n == 0 and h < 12:
                    w1_prep(4 + h)
                if h in (2, 4) and n + 1 < NB:
                    x_prep(n + 1, h // 2 - 1)
                if n == 0 and h in (13, 15):
                    w2_dma((h - 13) // 2)
                if n >= 1 and h % 2 == 1:
                    kt = 8 * (n - 1) + h // 2
                    if kt < HS:
                        w2_sign(kt)
                    if kt + 2 < HS:
                        w2_dma(kt + 2)
                l1_mm(n, h)
                if n == 0 and h == 2:
                    const_fill2()
                if n == 2 and h == 8:
                    # w3 contraction-major load (80KB, strided; hidden in
                    # L1 slack on the idle gpsimd queue)
                    nc.gpsimd.dma_start(
                        w3sb[:], w3[:].rearrange("(k p) o -> p k o", p=P)
                    )

        # ---- w3 binarize (load already done) ----
        nc.scalar.activation(w3bT[:, :, :O], w3sb[:], AF.Sign)

        # ---- layer 2 with layer-3 DoubleRow matmuls interleaved (lagged
        # two o-tiles so the Sign drains stay off the PE critical path) ----
        h2b = big.tile([P, HS, B_LOC], F8, tag="bigA", name="h2b")
        for n in range(NB):
            nsl = slice(n * NF, (n + 1) * NF)
            p3 = psum.tile([P, NF], F32, tag="l3", bufs=1, name="p3")

            def l3_pair(kk, n=n, nsl=nsl, p3=p3):
                ksl = slice(2 * kk, 2 * kk + 2)
                nc.tensor.matmul(
                    p3[:O, :],
                    w3bT[:, ksl, :O],
                    h2b[:, ksl, nsl],
                    start=(kk == 0),
                    stop=(kk == HS // 2 - 1),
                    perf_mode=DR,
                )

            for o in range(HS):
                if o >= 3 and o % 2 == 1:
                    l3_pair((o - 3) // 2)
                osl = slice(o * P, (o + 1) * P)
                pmm = psum.tile([P, NF], F32, tag="mm", bufs=5, name="pmm")
                for kk in range(HS // 2):
                    ksl = slice(2 * kk, 2 * kk + 2)
                    nc.tensor.matmul(
                        pmm[:],
                        w2bT[:, ksl, osl],
                        h1b[:, ksl, nsl],
                        start=(kk == 0),
                        stop=(kk == HS // 2 - 1),
                        perf_mode=DR,
                    )
                nc.scalar.activation(
                    h2b[:, o, nsl],
                    pmm[:],
                    AF.Sign,
                    bias=c2[:, o : o + 1],
                    scale=inv2[:, o : o + 1],
                )
            l3_pair(7)
            outsb = stage.tile([O, NF], F32, tag="outsb", name="outsb")
            nc.vector.tensor_scalar(
                outsb[:], p3[:O, :], b3sb[:], s10[:], ALU.add, ALU.mult
            )
            nc.sync.dma_start(out[:, nsl], outsb[:])

    nc.finalize()
    return nc


_CACHE = {}


def _get_nc():
    if "nc" not in _CACHE:
        _CACHE["nc"] = _build()
    return _CACHE["nc"]


def _in_maps(x, w1, b1, g1, be1, m1, v1, w2, b2, g2, be2, m2, v2, w3, b3, scale):
    f = lambda a: np.ascontiguousarray(np.asarray(a, dtype=np.float32))
    x2 = np.asarray(x, dtype=np.float32).reshape(B, D_IN)
    base = {
        "w1": f(np.asarray(w1, dtype=np.float32).T),
        "b1": f(b1),
        "g1": f(g1),
        "be1": f(be1),
        "m1": f(m1),
        "v1": f(v1),
        "w2": f(np.asarray(w2, dtype=np.float32).T),
        "b2": f(b2),
        "g2": f(g2),
        "be2": f(be2),
        "m2": f(m2),
        "v2": f(v2),
        "w3": f(np.asarray(w3, dtype=np.float32).T),
        "b3": f(b3),
        "scale": f(scale).reshape(1),
    }
    maps = []
    for c in range(N_CORES):
        m = dict(base)
        m["x"] = np.ascontiguousarray(x2[c * B_LOC : (c + 1) * B_LOC].T)
        maps.append(m)
    return maps


def _ensure_ntff_hook():
    """The agent image's antenv package lacks axon_hooks; synthesize it so
    run_bass_kernel_spmd's trace path can reach the axon NTFF profiler."""
    import sys
    import types

    if "antenv.axon_hooks" in sys.modules:
        return
    mod = types.ModuleType("antenv.axon_hooks")
    mod._hook = None

    def set_axon_ntff_profile_hook(h):
        mod._hook = h

    def get_axon_ntff_profile_hook():
        return mod._hook

    mod.set_axon_ntff_profile_hook = set_axon_ntff_profile_hook
    mod.get_axon_ntff_profile_hook = get_axon_ntff_profile_hook
    sys.modules["antenv.axon_hooks"] = mod
    import antenv

    antenv.axon_hooks = mod
    try:
        from trn_agent_boot.trn_boot import _ntff_profile_via_ctypes

        mod._hook = _ntff_profile_via_ctypes("/opt/axon/libaxon_pjrt.so")
    except Exception as e:
        print(f"ntff hook unavailable: {e}")


def run(trace=False, **inputs):
    if trace:
        _ensure_ntff_hook()
    nc = _get_nc()
    res = run_bass_kernel_spmd(
        nc, _in_maps(**inputs), core_ids=list(range(N_CORES)), trace=trace
    )
    outs = [r["out"] for r in res.results]
    full = np.concatenate([o.T for o in outs], axis=0).astype(np.float32)
    return full, res


def kernel(**inputs):
    return run(trace=False, **inputs)[0]


# revision 81
# speedup vs baseline: 1.0200x; 1.0111x over previous
"""Trainium2 Bass kernel for the BNN FASHION FC problem.

Network (per reference):
  h = x.reshape(B, 784)
  L1: h @ binarize(w1).T + b1 -> BN -> clip -> binarize     [B, 2048]
  L2: h @ binarize(w2).T + b2 -> BN -> clip -> binarize     [B, 2048]
  L3: (h @ binarize(w3).T + b3) * scale                     [B, 10]

Strategy (8 NeuronCores, data-parallel over batch, weights replicated):
  - Everything is computed with the hidden index on SBUF/PSUM partitions,
    so batchnorm affine + sign folds into one ScalarE activation (Sign
    with per-partition scale/bias) and layer N's output lands exactly in
    the [k=hidden, free=batch] layout layer N+1 needs.
  - The host-side sharding uploads x / w1 / w2 / w3 in contraction-major
    (transposed) layout, so layout prep on-device is DMA + Sign / hi-lo
    split only -- no PE transposes.  Binarization and the x split stay
    on device.
  - L1 splits x = hi + lo. hi runs in fp16 over 7 k-tiles, with the
    packed tail tile carrying hi rows 0:16 and an exact fp16 lo-tail in
    rows 16:32 (both against +-1 fp16 weights).  lo for the six full
    k-tiles runs in fp8e4m3 scaled by 2^14 against +-2^-14 fp8e5m2
    weights in DoubleRow mode (3 matmuls).  10 matmuls/tile total vs 13
    for the exact fp16 hi/lo scheme; final rel err ~1.5e-2 (gate 2e-2).
  - L2/L3 operands are all +-1: exact in fp8e4, run with DoubleRow.
  - binarize(clip(y)) == binarize(y), so clip is dropped.
  - BN folding: y = mm*inv + c with inv = g/sqrt(v+eps), c = (b-m)*inv + be.
  - L3 DoubleRow matmuls are interleaved into the L2 o-loop (lagged so
    the Sign drains stay off the PE critical path), shortening the tail.

Output per core is [10, 2048] (hidden-major); the host transposes and
concatenates to the full [16384, 10].
"""

import numpy as np
from contextlib import ExitStack

try:
    import concourse.bass as bass
except ImportError:  # staged repo location
    import sys

    sys.path.insert(0, "/opt/trn_rl_repo")
    import concourse.bass as bass

import concourse.mybir as mybir
import concourse.tile as tile
from concourse import bacc
from concourse.bass_utils import run_bass_kernel_spmd


P = 128
N_CORES = 8
B = 16384
B_LOC = B // N_CORES  # 2048 batch rows per core
D_IN = 784
KT = 7  # k tiles for layer-1 hi pass (784 -> 896 with packed lo-tail)
KLO = 6  # k tiles for layer-1 fp8 lo pass (the 768 full columns)
DP = KT * P  # 896
H = 2048
HS = H // P  # 16 hidden subtiles
NF = 512  # matmul free dim (one PSUM bank)
NB = B_LOC // NF  # 4 batch chunks
O = 10
KTAIL = D_IN - KLO * P  # 16-row tail (784 = 6*128 + 16)
LO_SCALE = float(2.0**14)  # lo pass: data scaled up, weights scaled down
W_LO = float(2.0**-14)  # exactly representable (normal) in fp8e5m2

F32 = mybir.dt.float32
F16 = mybir.dt.float16
F8 = mybir.dt.float8e4
F8E5 = mybir.dt.float8e5
AF = mybir.ActivationFunctionType
ALU = mybir.AluOpType
DR = mybir.MatmulPerfMode.DoubleRow


def _build():
    nc = bacc.Bacc(trn_type="TRN2")

    def din(name, shape):
        return nc.dram_tensor(name, shape, F32, kind="ExternalInput")

    # x/w1/w2/w3 arrive contraction-major (pre-transposed by the host shard)
    x = din("x", [D_IN, B_LOC])
    w1 = din("w1", [D_IN, H])
    b1 = din("b1", [H])
    g1 = din("g1", [H])
    be1 = din("be1", [H])
    m1 = din("m1", [H])
    v1 = din("v1", [H])
    w2 = din("w2", [H, H])
    b2 = din("b2", [H])
    g2 = din("g2", [H])
    be2 = din("be2", [H])
    m2 = din("m2", [H])
    v2 = din("v2", [H])
    w3 = din("w3", [H, O])
    b3 = din("b3", [O])
    scale = din("scale", [1])
    out = nc.dram_tensor("out", [O, B_LOC], F32, kind="ExternalOutput")

    with ExitStack() as ctx:
        tc = ctx.enter_context(tile.TileContext(nc))
        consts = ctx.enter_context(tc.tile_pool(name="consts", bufs=1))
        big = ctx.enter_context(tc.tile_pool(name="big", bufs=1))
        stage = ctx.enter_context(tc.tile_pool(name="stage", bufs=2))
        wstage = ctx.enter_context(tc.tile_pool(name="wstage", bufs=2))
        psum = ctx.enter_context(tc.tile_pool(name="psum", bufs=1, space="PSUM"))

        # PE warm-up: dependency-free dummy matmuls (outputs never read).
        # They run right after the engine barrier while the first x/w1 tiles
        # are still loading, so the HAM clock gate is already at 8/8 when
        # the real matmuls arrive.
        warm_in = consts.tile([P, NF], F16, name="warm_in")
        nc.vector.memset(warm_in[:], 1.0)
        warm_ps = psum.tile([P, NF], F32, tag="mm", bufs=5, name="warm_ps")

        def warm(k, w=NF):
            for _ in range(k):
                nc.tensor.matmul(
                    warm_ps[:, :w], warm_in[:, :P], warm_in[:, :w], start=True, stop=True
                )

        warm(7)

        # ---- persistent big tensors ----
        # h2b shares the xThi slot (xThi dead once L1 is done); xThi padded
        # to 8 k-tiles so the slot is h2b-sized.
        xThi = big.tile([P, 8, B_LOC], F16, tag="bigA", name="xThi")
        xTlo = big.tile([P, KLO, B_LOC], F8, tag="bigB", name="xTlo")
        w1bT = big.tile([P, KT, H], F16, tag="bigC", name="w1bT")
        w1lT = big.tile([P, KLO, H], F8E5, tag="bigD", name="w1lT")
        w2bT = big.tile([P, HS, H], F8, tag="bigE", name="w2bT")
        h1b = big.tile([P, HS, B_LOC], F8, tag="bigF", name="h1b")

        # the packed-tail k-tile: rows 0:16 hi, 16:32 lo16, rest must be 0
        nc.vector.memset(xThi[:, KLO, :], 0.0)
        nc.vector.memset(w1bT[:, KLO, :], 0.0)

        def w1_prep(ht):
            """Load the [784, 128] w1 h-slice (contraction-major), binarize
            into w1bT (with the fp16 lo-tail weights duplicated in rows
            16:32 of k-tile 6) and derive the +-2^-14 fp8e5 lo weights."""
            hsl = slice(ht * P, (ht + 1) * P)
            w1sb = stage.tile([P, KLO, P], F32, tag="w1f32", bufs=2, name="w1sb")
            nc.sync.dma_start(
                w1sb[:], w1[: KLO * P, hsl].rearrange("(k p) h -> p k h", p=P)
            )
            nc.scalar.activation(w1bT[:, :KLO, hsl], w1sb[:], AF.Sign)
            nc.vector.tensor_scalar(
                w1lT[:, :, hsl], w1bT[:, :KLO, hsl], W_LO, None, ALU.mult
            )
            wts = stage.tile([KTAIL, P], F32, tag="w1tl", bufs=2, name="wts")
            nc.gpsimd.dma_start(wts[:], w1[KLO * P : D_IN, hsl])
            nc.scalar.activation(w1bT[:KTAIL, KLO, hsl], wts[:], AF.Sign)
            nc.scalar.activation(w1bT[32 : 32 + KTAIL, KLO, hsl], wts[:], AF.Sign)

        def x_prep(n, g):
            """Load one 3-k-tile group of a batch chunk of the contraction-
            major x, split hi (fp16) / lo (fp8e4 scaled 2^14) on DVE; group 1
            also handles the 16-row tail. No PE work at all."""
            nsl = slice(n * NF, (n + 1) * NF)
            ksl = slice(3 * g, 3 * g + 3)
            xsb = stage.tile([P, 3, NF], F32, tag="xf32", bufs=2, name="xsb")
            nc.sync.dma_start(
                xsb[:],
                x[3 * g * P : (3 * g + 3) * P, nsl].rearrange(
                    "(k p) b -> p k b", p=P
                ),
            )
            hi = xThi[:, ksl, nsl]
            nc.vector.tensor_copy(hi, xsb[:])
            t16 = stage.tile([P, 3, NF], F16, tag="xlo16", bufs=1, name="t16")
            nc.vector.tensor_tensor(t16[:], xsb[:], hi, ALU.subtract)
            nc.vector.tensor_scalar(
                xTlo[:, ksl, nsl], t16[:], LO_SCALE, None, ALU.mult
            )
            if g == 1:
                xts = stage.tile([KTAIL, NF], F32, tag="xtl", bufs=2, name="xts")
                nc.scalar.dma_start(xts[:], x[KLO * P : D_IN, nsl])
                hit = xThi[:KTAIL, KLO, nsl]
                nc.scalar.copy(hit, xts[:])
                nc.vector.tensor_tensor(
                    xThi[32 : 32 + KTAIL, KLO, nsl], xts[:], hit, ALU.subtract
                )

        w2_tiles = {}

        def w2_dma(kt):
            w2sb = wstage.tile([P, H], F32, tag="w2f32", bufs=3, name="w2sb")
            nc.sync.dma_start(w2sb[:], w2[kt * P : (kt + 1) * P, :])
            w2_tiles[kt] = w2sb

        def w2_sign(kt):
            # binarize on the (idle) DVE to keep ScalarE free for L1 drains:
            # (w >= 0)*2 in fp8, then -1 in place -> +-1
            t = w2_tiles.pop(kt)
            nc.vector.tensor_scalar(
                w2bT[:, kt, :], t[:], 0.0, 2.0, ALU.is_ge, ALU.mult
            )
            nc.vector.tensor_scalar_add(w2bT[:, kt, :], w2bT[:, kt, :], -1.0)

        def l1_mm(n, h):
            nsl = slice(n * NF, (n + 1) * NF)
            pmm = psum.tile([P, NF], F32, tag="mm", bufs=5, name="pmm")
            hsl = slice(h * P, (h + 1) * P)
            for k in range(KT):
                nc.tensor.matmul(
                    pmm[:], w1bT[:, k, hsl], xThi[:, k, nsl], start=(k == 0), stop=False
                )
            for kk in range(KLO // 2):
                ksl = slice(2 * kk, 2 * kk + 2)
                nc.tensor.matmul(
                    pmm[:],
                    w1lT[:, ksl, hsl],
                    xTlo[:, ksl, nsl],
                    start=False,
                    stop=(kk == KLO // 2 - 1),
                    perf_mode=DR,
                )
            nc.scalar.activation(
                h1b[:, h, nsl],
                pmm[:],
                AF.Sign,
                bias=c1[:, h : h + 1],
                scale=inv1[:, h : h + 1],
            )

        # BN-fold constant tiles are pre-declared so l1_mm drains can
        # reference them; layer-2 constants fill later in the pipeline.
        inv1 = consts.tile([P, HS], F32, name="inv1")
        c1 = consts.tile([P, HS], F32, name="c1")
        inv2 = consts.tile([P, HS], F32, name="inv2")
        c2 = consts.tile([P, HS], F32, name="c2")
        b3sb = consts.tile([O, 1], F32, name="b3sb")
        s10 = consts.tile([O, 1], F32, name="s10")
        w3bT = consts.tile([P, HS, 16], F8, name="w3bT")
        w3sb = consts.tile([P, HS, O], F32, name="w3sb")

        # ---- per-hidden-unit BN constants, laid out [p, subtile] ----
        def vec_sb(handle, name):
            tmp = stage.tile([HS, P], F32, tag="vtmp", bufs=2, name="vtmp")
            nc.gpsimd.dma_start(tmp[:], handle[:].rearrange("(s p) -> s p", p=P))
            ps = psum.tile([P, HS], F32, tag="tr", bufs=2, name="vtps")
            nc.tensor.transpose(ps[:], tmp[:], id32[:])
            t = consts.tile([P, HS], F32, name=name)
            nc.vector.tensor_copy(t[:], ps[:])
            return t

        def bn_fold(inv, c, gs, bes, ms, bs, vs):
            nc.vector.tensor_scalar_add(inv, vs, 1e-5)
            nc.scalar.activation(inv, inv, AF.Sqrt)
            nc.vector.reciprocal(inv, inv)
            nc.vector.tensor_mul(inv, gs, inv)
            nc.vector.tensor_sub(c, bs, ms)
            nc.vector.tensor_mul(c, c, inv)
            nc.vector.tensor_add(c, c, bes)

        def const_fill1():
            b1s = vec_sb(b1, "b1s")
            g1s = vec_sb(g1, "g1s")
            be1s = vec_sb(be1, "be1s")
            m1s = vec_sb(m1, "m1s")
            v1s = vec_sb(v1, "v1s")
            bn_fold(inv1, c1, g1s, be1s, m1s, b1s, v1s)

        def const_fill2():
            b2s = vec_sb(b2, "b2s")
            g2s = vec_sb(g2, "g2s")
            be2s = vec_sb(be2, "be2s")
            m2s = vec_sb(m2, "m2s")
            v2s = vec_sb(v2, "v2s")
            bn_fold(inv2, c2, g2s, be2s, m2s, b2s, v2s)
            nc.gpsimd.dma_start(b3sb[:], b3[:].rearrange("(o u) -> o u", u=1))
            for i in range(O):
                nc.gpsimd.dma_start(
                    s10[i : i + 1, :], scale[:].rearrange("(s u) -> s u", u=1)
                )

        # identity for the small BN-vector transposes
        from concourse.masks import make_identity

        id32 = consts.tile([HS, HS], F32, name="id32")
        make_identity(nc, id32)

        # ---- main pipeline over batch chunks, software-pipelined prep ----
        w1_prep(0)
        x_prep(0, 0)
        warm(4, w=P)
        x_prep(0, 1)
        warm(4, w=P)
        for t in range(1, 4):
            w1_prep(t)
            warm(3, w=P)
        const_fill1()
        for n in range(NB):
            for h in range(HS):
                if n == 0 and h < 12:
                    w1_prep(4 + h)
                if h in (2, 4) and n + 1 < NB:
                    x_prep(n + 1, h // 2 - 1)
                if n == 0 and h in (13, 15):
                    w2_dma((h - 13) // 2)
                if n >= 1 and h % 2 == 1:
                    kt = 8 * (n - 1) + h // 2
                    if kt < HS:
                        w2_sign(kt)
                    if kt + 2 < HS:
                        w2_dma(kt + 2)
                l1_mm(n, h)
                if n == 0 and h == 2:
                    const_fill2()
                if n == 2 and h == 8:
                    # w3 contraction-major load (80KB, strided; hidden in
                    # L1 slack on the idle gpsimd queue)
                    nc.gpsimd.dma_start(
                        w3sb[:], w3[:].rearrange("(k p) o -> p k o", p=P)
                    )

        # ---- w3 binarize (load already done) ----
        nc.scalar.activation(w3bT[:, :, :O], w3sb[:], AF.Sign)

        # ---- layer 2 with layer-3 DoubleRow matmuls interleaved (lagged
        # two o-tiles so the Sign drains stay off the PE critical path) ----
        h2b = big.tile([P, HS, B_LOC], F8, tag="bigA", name="h2b")
        for n in range(NB):
            nsl = slice(n * NF, (n + 1) * NF)
            p3 = psum.tile([P, NF], F32, tag="l3", bufs=1, name="p3")

            def l3_pair(kk, n=n, nsl=nsl, p3=p3):
                ksl = slice(2 * kk, 2 * kk + 2)
                nc.tensor.matmul(
                    p3[:O, :],
                    w3bT[:, ksl, :O],
                    h2b[:, ksl, nsl],
                    start=(kk == 0),
                    stop=(kk == HS // 2 - 1),
                    perf_mode=DR,
                )

            for o in range(HS):
                if o >= 3 and o % 2 == 1:
                    l3_pair((o - 3) // 2)
                osl = slice(o * P, (o + 1) * P)
                pmm = psum.tile([P, NF], F32, tag="mm", bufs=5, name="pmm")
                for kk in range(HS // 2):
                    ksl = slice(2 * kk, 2 * kk + 2)
                    nc.tensor.matmul(
                        pmm[:],
                        w2bT[:, ksl, osl],
                        h1b[:, ksl, nsl],
                        start=(kk == 0),
                        stop=(kk == HS // 2 - 1),
                        perf_mode=DR,
                    )
                nc.scalar.activation(
                    h2b[:, o, nsl],
                    pmm[:],
                    AF.Sign,
                    bias=c2[:, o : o + 1],
                    scale=inv2[:, o : o + 1],
                )
            l3_pair(7)
            outsb = stage.tile([O, NF], F32, tag="outsb", name="outsb")
            nc.vector.tensor_scalar(
                outsb[:], p3[:O, :], b3sb[:], s10[:], ALU.add, ALU.mult
            )
            nc.sync.dma_start(out[:, nsl], outsb[:])

    nc.finalize()
    return nc


_CACHE = {}


def _get_nc():
    if "nc" not in _CACHE:
        _CACHE["nc"] = _build()
    return _CACHE["nc"]


def _in_maps(x, w1, b1, g1, be1, m1, v1, w2, b2, g2, be2, m2, v2, w3, b3, scale):
    f = lambda a: np.ascontiguousarray(np.asarray(a, dtype=np.float32))
    x2 = np.asarray(x, dtype=np.float32).reshape(B, D_IN)
    base = {
        "w1": f(np.asarray(w1, dtype=np.float32).T),
        "b1": f(b1),
        "g1": f(g1),
        "be1": f(be1),
        "m1": f(m1),
        "v1": f(v1),
        "w2": f(np.asarray(w2, dtype=np.float32).T),
        "b2": f(b2),
        "g2": f(g2),
        "be2": f(be2),
        "m2": f(m2),
        "v2": f(v2),
        "w3": f(np.asarray(w3, dtype=np.float32).T),
        "b3": f(b3),
        "scale": f(scale).reshape(1),
    }
    maps = []
    for c in range(N_CORES):
        m = dict(base)
        m["x"] = np.ascontiguousarray(x2[c * B_LOC : (c + 1) * B_LOC].T)
        maps.append(m)
    return maps


def _ensure_ntff_hook():
    """The agent image's antenv package lacks axon_hooks; synthesize it so
    run_bass_kernel_spmd's trace path can reach the axon NTFF profiler."""
    import sys
    import types

    if "antenv.axon_hooks" in sys.modules:
        return
    mod = types.ModuleType("antenv.axon_hooks")
    mod._hook = None

    def set_axon_ntff_profile_hook(h):
        mod._hook = h

    def get_axon_ntff_profile_hook():
        return mod._hook

    mod.set_axon_ntff_profile_hook = set_axon_ntff_profile_hook
    mod.get_axon_ntff_profile_hook = get_axon_ntff_profile_hook
    sys.modules["antenv.axon_hooks"] = mod
    import antenv

    antenv.axon_hooks = mod
    try:
        from trn_agent_boot.trn_boot import _ntff_profile_via_ctypes

        mod._hook = _ntff_profile_via_ctypes("/opt/axon/libaxon_pjrt.so")
    except Exception as e:
        print(f"ntff hook unavailable: {e}")


def run(trace=False, **inputs):
    if trace:
        _ensure_ntff_hook()
    nc = _get_nc()
    res = run_bass_kernel_spmd(
        nc, _in_maps(**inputs), core_ids=list(range(N_CORES)), trace=trace
    )
    outs = [r["out"] for r in res.results]
    full = np.concatenate([o.T for o in outs], axis=0).astype(np.float32)
    return full, res


def kernel(**inputs):
    return run(trace=False, **inputs)[0]


# revision 82
# speedup vs baseline: 1.0218x; 1.0018x over previous
"""Trainium2 Bass kernel for the BNN FASHION FC problem.

Network (per reference):
  h = x.reshape(B, 784)
  L1: h @ binarize(w1).T + b1 -> BN -> clip -> binarize     [B, 2048]
  L2: h @ binarize(w2).T + b2 -> BN -> clip -> binarize     [B, 2048]
  L3: (h @ binarize(w3).T + b3) * scale                     [B, 10]

Strategy (8 NeuronCores, data-parallel over batch, weights replicated):
  - Everything is computed with the hidden index on SBUF/PSUM partitions,
    so batchnorm affine + sign folds into one ScalarE activation (Sign
    with per-partition scale/bias) and layer N's output lands exactly in
    the [k=hidden, free=batch] layout layer N+1 needs.
  - The host-side sharding uploads x / w1 / w2 / w3 in contraction-major
    (transposed) layout, so layout prep on-device is DMA + Sign / hi-lo
    split only -- no PE transposes.  Binarization and the x split stay
    on device.
  - L1 splits x = hi + lo. hi runs in fp16 over 7 k-tiles, with the
    packed tail tile carrying hi rows 0:16 and an exact fp16 lo-tail in
    rows 16:32 (both against +-1 fp16 weights).  lo for the six full
    k-tiles runs in fp8e4m3 scaled by 2^14 against +-2^-14 fp8e5m2
    weights in DoubleRow mode (3 matmuls).  10 matmuls/tile total vs 13
    for the exact fp16 hi/lo scheme; final rel err ~1.5e-2 (gate 2e-2).
  - L2/L3 operands are all +-1: exact in fp8e4, run with DoubleRow.
  - binarize(clip(y)) == binarize(y), so clip is dropped.
  - BN folding: y = mm*inv + c with inv = g/sqrt(v+eps), c = (b-m)*inv + be.
  - L3 DoubleRow matmuls are interleaved into the L2 o-loop (lagged so
    the Sign drains stay off the PE critical path), shortening the tail.

Output per core is [10, 2048] (hidden-major); the host transposes and
concatenates to the full [16384, 10].
"""

import numpy as np
from contextlib import ExitStack

try:
    import concourse.bass as bass
except ImportError:  # staged repo location
    import sys

    sys.path.insert(0, "/opt/trn_rl_repo")
    import concourse.bass as bass

import concourse.mybir as mybir
import concourse.tile as tile
from concourse import bacc
from concourse.bass_utils import run_bass_kernel_spmd


P = 128
N_CORES = 8
B = 16384
B_LOC = B // N_CORES  # 2048 batch rows per core
D_IN = 784
KT = 7  # k tiles for layer-1 hi pass (784 -> 896 with packed lo-tail)
KLO = 6  # k tiles for layer-1 fp8 lo pass (the 768 full columns)
DP = KT * P  # 896
H = 2048
HS = H // P  # 16 hidden subtiles
NF = 512  # matmul free dim (one PSUM bank)
NB = B_LOC // NF  # 4 batch chunks
O = 10
KTAIL = D_IN - KLO * P  # 16-row tail (784 = 6*128 + 16)
LO_SCALE = float(2.0**14)  # lo pass: data scaled up, weights scaled down
W_LO = float(2.0**-14)  # exactly representable (normal) in fp8e5m2

F32 = mybir.dt.float32
F16 = mybir.dt.float16
F8 = mybir.dt.float8e4
F8E5 = mybir.dt.float8e5
AF = mybir.ActivationFunctionType
ALU = mybir.AluOpType
DR = mybir.MatmulPerfMode.DoubleRow


def _build():
    nc = bacc.Bacc(trn_type="TRN2")

    def din(name, shape):
        return nc.dram_tensor(name, shape, F32, kind="ExternalInput")

    # x/w1/w2/w3 arrive contraction-major (pre-transposed by the host shard)
    x = din("x", [D_IN, B_LOC])
    w1 = din("w1", [D_IN, H])
    b1 = din("b1", [H])
    g1 = din("g1", [H])
    be1 = din("be1", [H])
    m1 = din("m1", [H])
    v1 = din("v1", [H])
    w2 = din("w2", [H, H])
    b2 = din("b2", [H])
    g2 = din("g2", [H])
    be2 = din("be2", [H])
    m2 = din("m2", [H])
    v2 = din("v2", [H])
    w3 = din("w3", [H, O])
    b3 = din("b3", [O])
    scale = din("scale", [1])
    out = nc.dram_tensor("out", [O, B_LOC], F32, kind="ExternalOutput")

    with ExitStack() as ctx:
        tc = ctx.enter_context(tile.TileContext(nc))
        consts = ctx.enter_context(tc.tile_pool(name="consts", bufs=1))
        big = ctx.enter_context(tc.tile_pool(name="big", bufs=1))
        stage = ctx.enter_context(tc.tile_pool(name="stage", bufs=2))
        wstage = ctx.enter_context(tc.tile_pool(name="wstage", bufs=2))
        psum = ctx.enter_context(tc.tile_pool(name="psum", bufs=1, space="PSUM"))

        # PE warm-up: dependency-free dummy matmuls (outputs never read).
        # They run right after the engine barrier while the first x/w1 tiles
        # are still loading, so the HAM clock gate is already at 8/8 when
        # the real matmuls arrive.
        warm_in = consts.tile([P, NF], F16, name="warm_in")
        nc.vector.memset(warm_in[:], 1.0)
        warm_ps = psum.tile([P, NF], F32, tag="mm", bufs=5, name="warm_ps")

        def warm(k, w=NF):
            for _ in range(k):
                nc.tensor.matmul(
                    warm_ps[:, :w], warm_in[:, :P], warm_in[:, :w], start=True, stop=True
                )

        warm(14)

        # ---- persistent big tensors ----
        # h2b shares the xThi slot (xThi dead once L1 is done); xThi padded
        # to 8 k-tiles so the slot is h2b-sized.
        xThi = big.tile([P, 8, B_LOC], F16, tag="bigA", name="xThi")
        xTlo = big.tile([P, KLO, B_LOC], F8, tag="bigB", name="xTlo")
        w1bT = big.tile([P, KT, H], F16, tag="bigC", name="w1bT")
        w1lT = big.tile([P, KLO, H], F8E5, tag="bigD", name="w1lT")
        w2bT = big.tile([P, HS, H], F8, tag="bigE", name="w2bT")
        h1b = big.tile([P, HS, B_LOC], F8, tag="bigF", name="h1b")

        # the packed-tail k-tile: rows 0:16 hi, 16:32 lo16, rest must be 0
        nc.vector.memset(xThi[:, KLO, :], 0.0)
        nc.vector.memset(w1bT[:, KLO, :], 0.0)

        def w1_prep(ht):
            """Load the [784, 128] w1 h-slice (contraction-major), binarize
            into w1bT (with the fp16 lo-tail weights duplicated in rows
            16:32 of k-tile 6) and derive the +-2^-14 fp8e5 lo weights."""
            hsl = slice(ht * P, (ht + 1) * P)
            w1sb = stage.tile([P, KLO, P], F32, tag="w1f32", bufs=2, name="w1sb")
            nc.sync.dma_start(
                w1sb[:], w1[: KLO * P, hsl].rearrange("(k p) h -> p k h", p=P)
            )
            nc.scalar.activation(w1bT[:, :KLO, hsl], w1sb[:], AF.Sign)
            nc.vector.tensor_scalar(
                w1lT[:, :, hsl], w1bT[:, :KLO, hsl], W_LO, None, ALU.mult
            )
            wts = stage.tile([KTAIL, P], F32, tag="w1tl", bufs=2, name="wts")
            nc.gpsimd.dma_start(wts[:], w1[KLO * P : D_IN, hsl])
            nc.scalar.activation(w1bT[:KTAIL, KLO, hsl], wts[:], AF.Sign)
            nc.scalar.activation(w1bT[32 : 32 + KTAIL, KLO, hsl], wts[:], AF.Sign)

        def x_prep(n, g):
            """Load one 3-k-tile group of a batch chunk of the contraction-
            major x, split hi (fp16) / lo (fp8e4 scaled 2^14) on DVE; group 1
            also handles the 16-row tail. No PE work at all."""
            nsl = slice(n * NF, (n + 1) * NF)
            ksl = slice(3 * g, 3 * g + 3)
            xsb = stage.tile([P, 3, NF], F32, tag="xf32", bufs=2, name="xsb")
            nc.sync.dma_start(
                xsb[:],
                x[3 * g * P : (3 * g + 3) * P, nsl].rearrange(
                    "(k p) b -> p k b", p=P
                ),
            )
            hi = xThi[:, ksl, nsl]
            nc.vector.tensor_copy(hi, xsb[:])
            t16 = stage.tile([P, 3, NF], F16, tag="xlo16", bufs=1, name="t16")
            nc.vector.tensor_tensor(t16[:], xsb[:], hi, ALU.subtract)
            nc.vector.tensor_scalar(
                xTlo[:, ksl, nsl], t16[:], LO_SCALE, None, ALU.mult
            )
            if g == 1:
                xts = stage.tile([KTAIL, NF], F32, tag="xtl", bufs=2, name="xts")
                nc.scalar.dma_start(xts[:], x[KLO * P : D_IN, nsl])
                hit = xThi[:KTAIL, KLO, nsl]
                nc.scalar.copy(hit, xts[:])
                nc.vector.tensor_tensor(
                    xThi[32 : 32 + KTAIL, KLO, nsl], xts[:], hit, ALU.subtract
                )

        w2_tiles = {}

        def w2_dma(kt):
            w2sb = wstage.tile([P, H], F32, tag="w2f32", bufs=3, name="w2sb")
            nc.sync.dma_start(w2sb[:], w2[kt * P : (kt + 1) * P, :])
            w2_tiles[kt] = w2sb

        def w2_sign(kt):
            # binarize on the (idle) DVE to keep ScalarE free for L1 drains:
            # (w >= 0)*2 in fp8, then -1 in place -> +-1
            t = w2_tiles.pop(kt)
            nc.vector.tensor_scalar(
                w2bT[:, kt, :], t[:], 0.0, 2.0, ALU.is_ge, ALU.mult
            )
            nc.vector.tensor_scalar_add(w2bT[:, kt, :], w2bT[:, kt, :], -1.0)

        def l1_mm(n, h):
            nsl = slice(n * NF, (n + 1) * NF)
            pmm = psum.tile([P, NF], F32, tag="mm", bufs=5, name="pmm")
            hsl = slice(h * P, (h + 1) * P)
            for k in range(KT):
                nc.tensor.matmul(
                    pmm[:], w1bT[:, k, hsl], xThi[:, k, nsl], start=(k == 0), stop=False
                )
            for kk in range(KLO // 2):
                ksl = slice(2 * kk, 2 * kk + 2)
                nc.tensor.matmul(
                    pmm[:],
                    w1lT[:, ksl, hsl],
                    xTlo[:, ksl, nsl],
                    start=False,
                    stop=(kk == KLO // 2 - 1),
                    perf_mode=DR,
                )
            nc.scalar.activation(
                h1b[:, h, nsl],
                pmm[:],
                AF.Sign,
                bias=c1[:, h : h + 1],
                scale=inv1[:, h : h + 1],
            )

        # BN-fold constant tiles are pre-declared so l1_mm drains can
        # reference them; layer-2 constants fill later in the pipeline.
        inv1 = consts.tile([P, HS], F32, name="inv1")
        c1 = consts.tile([P, HS], F32, name="c1")
        inv2 = consts.tile([P, HS], F32, name="inv2")
        c2 = consts.tile([P, HS], F32, name="c2")
        b3sb = consts.tile([O, 1], F32, name="b3sb")
        s10 = consts.tile([O, 1], F32, name="s10")
        w3bT = consts.tile([P, HS, 16], F8, name="w3bT")
        w3sb = consts.tile([P, HS, O], F32, name="w3sb")

        # ---- per-hidden-unit BN constants, laid out [p, subtile] ----
        def vec_sb(handle, name):
            tmp = stage.tile([HS, P], F32, tag="vtmp", bufs=2, name="vtmp")
            nc.gpsimd.dma_start(tmp[:], handle[:].rearrange("(s p) -> s p", p=P))
            ps = psum.tile([P, HS], F32, tag="tr", bufs=2, name="vtps")
            nc.tensor.transpose(ps[:], tmp[:], id32[:])
            t = consts.tile([P, HS], F32, name=name)
            nc.vector.tensor_copy(t[:], ps[:])
            return t

        def bn_fold(inv, c, gs, bes, ms, bs, vs):
            nc.vector.tensor_scalar_add(inv, vs, 1e-5)
            nc.scalar.activation(inv, inv, AF.Sqrt)
            nc.vector.reciprocal(inv, inv)
            nc.vector.tensor_mul(inv, gs, inv)
            nc.vector.tensor_sub(c, bs, ms)
            nc.vector.tensor_mul(c, c, inv)
            nc.vector.tensor_add(c, c, bes)

        def const_fill1():
            b1s = vec_sb(b1, "b1s")
            g1s = vec_sb(g1, "g1s")
            be1s = vec_sb(be1, "be1s")
            m1s = vec_sb(m1, "m1s")
            v1s = vec_sb(v1, "v1s")
            bn_fold(inv1, c1, g1s, be1s, m1s, b1s, v1s)

        def const_fill2():
            b2s = vec_sb(b2, "b2s")
            g2s = vec_sb(g2, "g2s")
            be2s = vec_sb(be2, "be2s")
            m2s = vec_sb(m2, "m2s")
            v2s = vec_sb(v2, "v2s")
            bn_fold(inv2, c2, g2s, be2s, m2s, b2s, v2s)
            nc.gpsimd.dma_start(b3sb[:], b3[:].rearrange("(o u) -> o u", u=1))
            for i in range(O):
                nc.gpsimd.dma_start(
                    s10[i : i + 1, :], scale[:].rearrange("(s u) -> s u", u=1)
                )

        # identity for the small BN-vector transposes
        from concourse.masks import make_identity

        id32 = consts.tile([HS, HS], F32, name="id32")
        make_identity(nc, id32)

        # ---- main pipeline over batch chunks, software-pipelined prep ----
        w1_prep(0)
        x_prep(0, 0)
        warm(4, w=P)
        x_prep(0, 1)
        warm(4, w=P)
        for t in range(1, 4):
            w1_prep(t)
            warm(3, w=P)
        const_fill1()
        for n in range(NB):
            for h in range(HS):
                if n == 0 and h < 12:
                    w1_prep(4 + h)
                if h in (2, 4) and n + 1 < NB:
                    x_prep(n + 1, h // 2 - 1)
                if n == 0 and h in (13, 15):
                    w2_dma((h - 13) // 2)
                if n >= 1 and h % 2 == 1:
                    kt = 8 * (n - 1) + h // 2
                    if kt < HS:
                        w2_sign(kt)
                    if kt + 2 < HS:
                        w2_dma(kt + 2)
                l1_mm(n, h)
                if n == 0 and h == 2:
                    const_fill2()
                if n == 2 and h == 8:
                    # w3 contraction-major load (80KB, strided; hidden in
                    # L1 slack on the idle gpsimd queue)
                    nc.gpsimd.dma_start(
                        w3sb[:], w3[:].rearrange("(k p) o -> p k o", p=P)
                    )

        # ---- w3 binarize (load already done) ----
        nc.scalar.activation(w3bT[:, :, :O], w3sb[:], AF.Sign)

        # ---- layer 2 with layer-3 DoubleRow matmuls interleaved (lagged
        # two o-tiles so the Sign drains stay off the PE critical path) ----
        h2b = big.tile([P, HS, B_LOC], F8, tag="bigA", name="h2b")
        for n in range(NB):
            nsl = slice(n * NF, (n + 1) * NF)
            p3 = psum.tile([P, NF], F32, tag="l3", bufs=1, name="p3")

            def l3_pair(kk, n=n, nsl=nsl, p3=p3):
                ksl = slice(2 * kk, 2 * kk + 2)
                nc.tensor.matmul(
                    p3[:O, :],
                    w3bT[:, ksl, :O],
                    h2b[:, ksl, nsl],
                    start=(kk == 0),
                    stop=(kk == HS // 2 - 1),
                    perf_mode=DR,
                )

            for o in range(HS):
                if o >= 3 and o % 2 == 1:
                    l3_pair((o - 3) // 2)
                osl = slice(o * P, (o + 1) * P)
                pmm = psum.tile([P, NF], F32, tag="mm", bufs=5, name="pmm")
                for kk in range(HS // 2):
                    ksl = slice(2 * kk, 2 * kk + 2)
                    nc.tensor.matmul(
                        pmm[:],
                        w2bT[:, ksl, osl],
                        h1b[:, ksl, nsl],
                        start=(kk == 0),
                        stop=(kk == HS // 2 - 1),
                        perf_mode=DR,
                    )
                nc.scalar.activation(
                    h2b[:, o, nsl],
                    pmm[:],
                    AF.Sign,
                    bias=c2[:, o : o + 1],
                    scale=inv2[:, o : o + 1],
                )
            l3_pair(7)
            outsb = stage.tile([O, NF], F32, tag="outsb", name="outsb")
            nc.vector.tensor_scalar(
                outsb[:], p3[:O, :], b3sb[:], s10[:], ALU.add, ALU.mult
            )
            nc.sync.dma_start(out[:, nsl], outsb[:])

    nc.finalize()
    return nc


_CACHE = {}


def _get_nc():
    if "nc" not in _CACHE:
        _CACHE["nc"] = _build()
    return _CACHE["nc"]


def _in_maps(x, w1, b1, g1, be1, m1, v1, w2, b2, g2, be2, m2, v2, w3, b3, scale):
    f = lambda a: np.ascontiguousarray(np.asarray(a, dtype=np.float32))
    x2 = np.asarray(x, dtype=np.float32).reshape(B, D_IN)
    base = {
        "w1": f(np.asarray(w1, dtype=np.float32).T),
        "b1": f(b1),
        "g1": f(g1),
        "be1": f(be1),
        "m1": f(m1),
        "v1": f(v1),
        "w2": f(np.asarray(w2, dtype=np.float32).T),
        "b2": f(b2),
        "g2": f(g2),
        "be2": f(be2),
        "m2": f(m2),
        "v2": f(v2),
        "w3": f(np.asarray(w3, dtype=np.float32).T),
        "b3": f(b3),
        "scale": f(scale).reshape(1),
    }
    maps = []
    for c in range(N_CORES):
        m = dict(base)
        m["x"] = np.ascontiguousarray(x2[c * B_LOC : (c + 1) * B_LOC].T)
        maps.append(m)
    return maps


def _ensure_ntff_hook():
    """The agent image's antenv package lacks axon_hooks; synthesize it so
    run_bass_kernel_spmd's trace path can reach the axon NTFF profiler."""
    import sys
    import types

    if "antenv.axon_hooks" in sys.modules:
        return
    mod = types.ModuleType("antenv.axon_hooks")
    mod._hook = None

    def set_axon_ntff_profile_hook(h):
        mod._hook = h

    def get_axon_ntff_profile_hook():
        return mod._hook

    mod.set_axon_ntff_profile_hook = set_axon_ntff_profile_hook
    mod.get_axon_ntff_profile_hook = get_axon_ntff_profile_hook
    sys.modules["antenv.axon_hooks"] = mod
    import antenv

    antenv.axon_hooks = mod
    try:
        from trn_agent_boot.trn_boot import _ntff_profile_via_ctypes

        mod._hook = _ntff_profile_via_ctypes("/opt/axon/libaxon_pjrt.so")
    except Exception as e:
        print(f"ntff hook unavailable: {e}")


def run(trace=False, **inputs):
    if trace:
        _ensure_ntff_hook()
    nc = _get_nc()
    res = run_bass_kernel_spmd(
        nc, _in_maps(**inputs), core_ids=list(range(N_CORES)), trace=trace
    )
    outs = [r["out"] for r in res.results]
    full = np.concatenate([o.T for o in outs], axis=0).astype(np.float32)
    return full, res


def kernel(**inputs):
    return run(trace=False, **inputs)[0]
